# revision 13
# baseline (speedup 1.0000x reference)
"""Trainium2 kernel for: out = (mat1 @ mat2 + input_tensor).astype(f32), all int32 in [0,16).

Strategy
--------
Values are integers in [0, 15], so:
  - mat1/mat2 are exact in fp8 e4m3 (integers 0..15 need 4 significand bits; e4m3 has 4)
  - products (<= 225) are exact in the PE datapath (e6m3 upcast -> e10m10 product)
  - accumulators (<= 15*15*4096 + 15 = 921,615 < 2^24) are exact in fp32 PSUM
so an fp8 DoubleRow matmul (2 MACs/cell/cycle, the fastest PE mode on trn2)
reproduces the int32 reference bit-exactly in fp32.

Sharding: 2D, 4 mat1-row blocks x 2 mat2-column blocks over 8 cores. Each
core computes a [1024, 2048] slab of the output. Pure SPMD, no collectives.

Per-core device program (512 DoubleRow matmuls, each [K=256]x[FD=512] = the
fp8 ALU roofline of 216 ns/MM; stream floor ~110.6 us):
  - m1 is packed kt-major on the host ([KT, P, MT, 2, P]) and streamed one
    kt-slice (2 KiB/partition) per DMA on the *Scalar* HW-DGE ring, while m2
    streams on the *Sync* ring -- two rings pay their ~1.5 us startup in
    parallel and the first real matmul needs only m1[kt=0] + m2[kt=0..1]
    (384 KiB), so the stream starts ~9.5 us instead of ~13.8 us.
  - nb=0 runs kt-outer/mt-inner: one kt sweep (8 MMs, 1.73 us) consumes
    256+128 KiB, a 222 GB/s pace the DMA sustains from cold. All 8 PSUM
    banks accumulate across the kt loop. The PE warmup is only ~20
    throwaway matmuls -- just enough to keep the HAM activity window busy
    (cold 1.2 GHz -> warm 2.4 GHz at ~4 us) until the first data lands.
  - nb=1..3 run mt-outer (m2 fully prefetched): per m-tile, 16 DoubleRow
    MMs accumulate a [128, 512] PSUM bank; DVE adds the fp8 input_tensor
    during PSUM->SBUF eviction; stores issue on Sync (all input loads are
    queued ahead of any store, so a waiting store can't head-of-line-block
    a load).
  - the final m-tile of the final nb evicts in four [128,128] quarters with
    store issues alternating Sync/Scalar, shortening the last-MM -> last-DMA
    drain chain to ~1.5 us.
"""

import numpy as np
import ml_dtypes

import concourse.bass as bass
import concourse.mybir as mybir
import concourse.tile as tile
from concourse import bacc
from concourse.bass import ts
from concourse.bass_utils import run_bass_kernel_spmd

F8 = mybir.dt.float8e4
F32 = mybir.dt.float32

N_CORES = 8
A_SHARD = 4  # mat1 row blocks
B_SHARD = 2  # mat2 col blocks
P = 128  # partitions
NB_TILE = 512  # output free-dim tile (one PSUM bank of fp32)
KP = 256  # contraction per DoubleRow matmul (2 x 128)
N_WARMUP = 46  # HAM-warming throwaway matmuls (~107 ns each, cold)


def build_program(m_shard: int, K: int, n_shard: int) -> bass.Bass:
    """One NeuronCore's program: [m_shard, K] @ [K, n_shard] + input -> fp32.

    DRAM parameter layouts (host pre-packs; p is the SBUF partition index):
      m1  [KT, P, MT, 2, P] fp8    : m1[kt, p, mt, i, m] = mat1_blk[P*mt + m, KP*kt + 128*i + p]
      m2  [NB, P, KT, 2, 512] fp8  : m2[nb, p, kt, i, n] = mat2_blk[KP*kt + 128*i + p, 512*nb + n]
      inp [NB, P, MT, 512] fp8     : inp[nb, p, mt, n] = input_blk[P*mt + p, 512*nb + n]
      out [NB, P, MT, 512] f32     : out[nb, p, mt, n] = result[P*mt + p, 512*nb + n]
    """
    KT = K // KP
    MT = m_shard // P
    NB = n_shard // NB_TILE
    assert NB == 4, "DMA ring layout below is written for NB == 4"

    nc = bacc.Bacc("TRN2", target_bir_lowering=False, debug=False)
    m1d = nc.dram_tensor("m1", [KT, P, MT, 2, P], F8, kind="ExternalInput")
    m2d = nc.dram_tensor("m2", [NB, P, KT, 2, NB_TILE], F8, kind="ExternalInput")
    inpd = nc.dram_tensor("inp", [NB, P, MT, NB_TILE], F8, kind="ExternalInput")
    outd = nc.dram_tensor("out", [NB, P, MT, NB_TILE], F32, kind="ExternalOutput")

    with tile.TileContext(nc) as tc:
        with (
            tc.tile_pool(name="m1", bufs=1) as m1_pool,
            tc.tile_pool(name="m2", bufs=3) as m2_pool,
            tc.tile_pool(name="inp", bufs=4) as inp_pool,
            tc.tile_pool(name="res", bufs=2) as res_pool,
            tc.tile_pool(name="psum", bufs=8, space="PSUM") as psum_pool,
        ):
            # All of m1, kt-major, streamed per kt-slice on the Scalar ring
            # (2 KiB/partition each) so slice kt is usable as soon as it
            # lands; the Sync ring carries m2/inp in parallel.
            m1s = m1_pool.tile([P, KT, MT, 2, P], F8, name="m1s", tag="m1", bufs=1)
            # kt=0 in two half-slices so the first matmuls gate on 128 KiB.
            nc.scalar.dma_start(m1s[:, 0, : MT // 2], m1d[0, :, : MT // 2])
            nc.scalar.dma_start(m1s[:, 0, MT // 2 :], m1d[0, :, MT // 2 :])
            for kt in range(1, KT):
                nc.scalar.dma_start(m1s[:, kt], m1d[kt])

            # PE warmup on a zeroed tile: keeps the HAM activity window busy
            # from ~7.4 us so the clock is ramping while the first DMAs land.
            warm_src = inp_pool.tile([P, P], F8, tag="warm", bufs=1)
            nc.gpsimd.memset(warm_src[:], 0.0)
            warm_ps = psum_pool.tile([P, NB_TILE], F32, tag="ps")
            for _ in range(N_WARMUP):
                nc.tensor.matmul(
                    warm_ps[:, :P], warm_src[:], warm_src[:], start=True, stop=True
                )

            # Each DMA ring delivers strictly in queue order, and the two
            # rings split bandwidth ~evenly with no QoS — so every ring's
            # queue is ordered by consumption time, and bulk prefetch only
            # sits behind data that is needed earlier on the same ring.
            #   ring A (Scalar): m1 kt-slices, then m2[nb=1] chunks, m2[3]
            #   ring B (Sync):   m2[nb=0] chunks, inp0, m2[2], inp1..3, stores
            m2_tiles = [
                m2_pool.tile([P, KT, 2, NB_TILE], F8, tag="m2", name=f"m2s{nb}")
                for nb in range(NB)
            ]
            nc.sync.dma_start(m2_tiles[0][:, 0:1], m2d[0, :, 0:1])
            nc.sync.dma_start(m2_tiles[0][:, 1:2], m2d[0, :, 1:2])
            for k0 in range(2, KT, 2):
                nc.sync.dma_start(m2_tiles[0][:, k0 : k0 + 2], m2d[0, :, k0 : k0 + 2])
            inps_all = [
                inp_pool.tile([P, MT, NB_TILE], F8, tag="inp", name=f"inps{nb}")
                for nb in range(NB)
            ]
            nc.sync.dma_start(inps_all[0][:], inpd[0])
            nc.sync.dma_start(m2_tiles[2][:], m2d[2])
            for nb in range(1, NB):
                nc.sync.dma_start(inps_all[nb][:], inpd[nb])
            for k0 in range(0, KT, 2):
                nc.scalar.dma_start(
                    m2_tiles[1][:, k0 : k0 + 2], m2d[1, :, k0 : k0 + 2]
                )
            nc.scalar.dma_start(m2_tiles[3][:], m2d[3])

            def evict(nb, mt, pss, inps, outs):
                """PSUM -> SBUF (+input) on DVE, then store (Sync ring)."""
                nc.vector.tensor_add(outs[:, mt], pss[mt][:], inps[:, mt])
                nc.sync.dma_start(outd[nb, :, mt], outs[:, mt])

            # ---- nb = 0..NB-2: kt-outer / mt-inner -------------------------
            # One kt sweep = 8 MMs consuming one m1 kt-slice + one m2
            # kt-slice (384 KiB / 1.73 us = 222 GB/s), a pace the rings
            # sustain from cold; the stream starts as soon as (m1[kt=0],
            # m2[kt=0]) land. All 8 PSUM banks accumulate across the kt
            # loop; at an nb boundary the evictions chase the final kt
            # sweep m-tile by m-tile, so the next nb's matmuls never wait.
            for nb in range(NB - 1):
                m2s = m2_tiles[nb]
                pss = [psum_pool.tile([P, NB_TILE], F32, name=f"ps{nb}_{mt}", tag="ps")
                       for mt in range(MT)]
                outs = res_pool.tile([P, MT, NB_TILE], F32)
                for kt in range(KT):
                    for mt in range(MT):
                        nc.tensor.matmul(
                            pss[mt][:],
                            m1s[:, kt, mt],
                            m2s[:, kt],
                            start=(kt == 0),
                            stop=(kt == KT - 1),
                            perf_mode=mybir.MatmulPerfMode.DoubleRow,
                        )
                for mt in range(MT):
                    evict(nb, mt, pss, inps_all[nb], outs)

            # ---- nb = NB-1: mt-outer (m2 long prefetched) ------------------
            if True:
                nb = NB - 1
                m2s = m2_tiles[nb]
                inps = inps_all[nb]
                outs = res_pool.tile([P, MT, NB_TILE], F32)
                pss = [psum_pool.tile([P, NB_TILE], F32, name=f"ps{nb}_{mt}", tag="ps")
                       for mt in range(MT - 1)]
                last_mt = MT - 1
                for mt in range(MT - 1):
                    for kt in range(KT):
                        nc.tensor.matmul(
                            pss[mt][:],
                            m1s[:, kt, mt],
                            m2s[:, kt],
                            start=(kt == 0),
                            stop=(kt == KT - 1),
                            perf_mode=mybir.MatmulPerfMode.DoubleRow,
                        )
                    evict(nb, mt, pss, inps, outs)
                # Final m-tile: column-split across two PSUM banks. Half A
                # (cols 0..255) finishes 16 MMs early and its add+store
                # (Sync) overlap half B's matmuls; after the very last MM
                # only B's [128,256] add + one Scalar-ring store remain.
                H = NB_TILE // 2
                psA = psum_pool.tile([P, H], F32, name="psA", tag="ps")
                psB = psum_pool.tile([P, H], F32, name="psB", tag="ps")
                for ps, h in ((psA, 0), (psB, 1)):
                    cs = slice(h * H, (h + 1) * H)
                    for kt in range(KT):
                        nc.tensor.matmul(
                            ps[:],
                            m1s[:, kt, last_mt],
                            m2s[:, kt, :, cs],
                            start=(kt == 0),
                            stop=(kt == KT - 1),
                            perf_mode=mybir.MatmulPerfMode.DoubleRow,
                        )
                    nc.vector.tensor_add(
                        outs[:, last_mt, cs], ps[:], inps[:, last_mt, cs]
                    )
                    eng = nc.sync if h == 0 else nc.scalar
                    eng.dma_start(outd[nb, :, last_mt, cs], outs[:, last_mt, cs])
    nc.compile()
    return nc


def pack_m1_block(blk: np.ndarray) -> np.ndarray:
    """[m_shard, K] int -> [KT, P, MT, 2, P] fp8 (kt-major DoubleRow weights)."""
    m_shard, K = blk.shape
    # [mt, m, kt, i, p] from blk[P*mt + m, KP*kt + 128*i + p]
    r = blk.reshape(m_shard // P, P, K // KP, 2, P)
    return np.ascontiguousarray(r.transpose(2, 4, 0, 3, 1)).astype(np.float32).astype(
        ml_dtypes.float8_e4m3
    )


def pack_m2(mat2: np.ndarray) -> np.ndarray:
    """[K, N] int -> [N//512, P, KT, 2, 512] fp8 (DoubleRow moving layout)."""
    K, N = mat2.shape
    r = mat2.reshape(K // KP, 2, P, N // NB_TILE, NB_TILE)  # [kt, i, p, nb, n]
    return np.ascontiguousarray(r.transpose(3, 2, 0, 1, 4)).astype(np.float32).astype(
        ml_dtypes.float8_e4m3
    )


def pack_inp_block(blk: np.ndarray) -> np.ndarray:
    """[m_shard, n_shard] int -> [NB, P, MT, 512] fp8 (0..15 are exact)."""
    m_shard, n_shard = blk.shape
    r = blk.reshape(m_shard // P, P, n_shard // NB_TILE, NB_TILE)  # [mt, p, nb, n]
    return (
        np.ascontiguousarray(r.transpose(2, 1, 0, 3))
        .astype(np.float32)
        .astype(ml_dtypes.float8_e4m3)
    )


def unpack_out(packed: np.ndarray, m_shard: int, n_shard: int) -> np.ndarray:
    """[NB, P, MT, 512] f32 -> [m_shard, n_shard] f32."""
    return np.ascontiguousarray(packed.transpose(2, 1, 0, 3)).reshape(m_shard, n_shard)


def _prepare(input_tensor, mat1, mat2):
    input_tensor = np.asarray(input_tensor)
    mat1 = np.asarray(mat1)
    mat2 = np.asarray(mat2)
    M, K = mat1.shape
    N = mat2.shape[1]
    m_shard = M // A_SHARD
    n_shard = N // B_SHARD
    nb_per_core = n_shard // NB_TILE

    nc = build_program(m_shard, K, n_shard)

    m2p = pack_m2(mat2)  # [N//512, P, KT, 2, 512]; core takes its nb range
    in_maps = []
    for c in range(N_CORES):
        ra, cb = divmod(c, B_SHARD)
        rows = slice(ra * m_shard, (ra + 1) * m_shard)
        cols = slice(cb * n_shard, (cb + 1) * n_shard)
        nbs = slice(cb * nb_per_core, (cb + 1) * nb_per_core)
        in_maps.append(
            {
                "m1": pack_m1_block(mat1[rows]),
                "m2": m2p[nbs],
                "inp": pack_inp_block(input_tensor[rows, cols]),
            }
        )
    return nc, in_maps, (m_shard, n_shard)


def _gather(results, m_shard, n_shard):
    M = m_shard * A_SHARD
    N = n_shard * B_SHARD
    out = np.empty((M, N), dtype=np.float32)
    for c in range(N_CORES):
        ra, cb = divmod(c, B_SHARD)
        out[
            ra * m_shard : (ra + 1) * m_shard, cb * n_shard : (cb + 1) * n_shard
        ] = unpack_out(results[c]["out"], m_shard, n_shard)
    return out


def kernel(input_tensor, mat1, mat2):
    nc, in_maps, (m_shard, n_shard) = _prepare(input_tensor, mat1, mat2)
    res = run_bass_kernel_spmd(nc, in_maps, list(range(N_CORES))).results
    return _gather(res, m_shard, n_shard)


def kernel_traced(input_tensor, mat1, mat2, **kwargs):
    """Like kernel(), but also returns BassKernelResults (exec_time_ns etc.)."""
    nc, in_maps, (m_shard, n_shard) = _prepare(input_tensor, mat1, mat2)
    res = run_bass_kernel_spmd(
        nc, in_maps, list(range(N_CORES)), trace=True, **kwargs
    )
    return _gather(res.results, m_shard, n_shard), res


# revision 16
# speedup vs baseline: 1.0425x; 1.0425x over previous
"""Trainium2 kernel for: out = (mat1 @ mat2 + input_tensor).astype(f32), all int32 in [0,16).

Strategy
--------
Values are integers in [0, 15], so:
  - mat1/mat2 are exact in fp8 e4m3 (integers 0..15 need 4 significand bits; e4m3 has 4)
  - products (<= 225) are exact in the PE datapath (e6m3 upcast -> e10m10 product)
  - accumulators (<= 15*15*4096 + 15 = 921,615 < 2^24) are exact in fp32 PSUM
so an fp8 DoubleRow matmul (2 MACs/cell/cycle, the fastest PE mode on trn2)
reproduces the int32 reference bit-exactly in fp32.

Sharding: 2D, 4 mat1-row blocks x 2 mat2-column blocks over 8 cores. Each
core computes a [1024, 2048] slab of the output. Pure SPMD, no collectives.

Per-core device program (512 DoubleRow matmuls, each [K=256]x[FD=512] = the
fp8 ALU roofline of 216 ns/MM; stream floor ~110.6 us):
  - m1 is packed kt-major on the host ([KT, P, MT, 2, P]) and streamed one
    kt-slice (2 KiB/partition) per DMA on the *Scalar* HW-DGE ring, while m2
    streams on the *Sync* ring -- two rings pay their ~1.5 us startup in
    parallel and the first real matmul needs only m1[kt=0] + m2[kt=0..1]
    (384 KiB), so the stream starts ~9.5 us instead of ~13.8 us.
  - nb=0 runs kt-outer/mt-inner: one kt sweep (8 MMs, 1.73 us) consumes
    256+128 KiB, a 222 GB/s pace the DMA sustains from cold. All 8 PSUM
    banks accumulate across the kt loop. The PE warmup is only ~20
    throwaway matmuls -- just enough to keep the HAM activity window busy
    (cold 1.2 GHz -> warm 2.4 GHz at ~4 us) until the first data lands.
  - nb=1..3 run mt-outer (m2 fully prefetched): per m-tile, 16 DoubleRow
    MMs accumulate a [128, 512] PSUM bank; DVE adds the fp8 input_tensor
    during PSUM->SBUF eviction; stores issue on Sync (all input loads are
    queued ahead of any store, so a waiting store can't head-of-line-block
    a load).
  - the final m-tile of the final nb evicts in four [128,128] quarters with
    store issues alternating Sync/Scalar, shortening the last-MM -> last-DMA
    drain chain to ~1.5 us.
"""

import numpy as np
import ml_dtypes

import concourse.bass as bass
import concourse.mybir as mybir
import concourse.tile as tile
from concourse import bacc
from concourse.bass import ts
from concourse.bass_utils import run_bass_kernel_spmd

F8 = mybir.dt.float8e4
F32 = mybir.dt.float32

N_CORES = 8
A_SHARD = 4  # mat1 row blocks
B_SHARD = 2  # mat2 col blocks
P = 128  # partitions
NB_TILE = 512  # output free-dim tile (one PSUM bank of fp32)
KP = 256  # contraction per DoubleRow matmul (2 x 128)
N_WARMUP = 38  # HAM-warming throwaway matmuls (~107 ns each, cold)


def build_program(m_shard: int, K: int, n_shard: int) -> bass.Bass:
    """One NeuronCore's program: [m_shard, K] @ [K, n_shard] + input -> fp32.

    DRAM parameter layouts (host pre-packs; p is the SBUF partition index):
      m1  [KT, P, MT, 2, P] fp8    : m1[kt, p, mt, i, m] = mat1_blk[P*mt + m, KP*kt + 128*i + p]
      m2  [NB, P, KT, 2, 512] fp8  : m2[nb, p, kt, i, n] = mat2_blk[KP*kt + 128*i + p, 512*nb + n]
      inp [NB, P, MT, 512] fp8     : inp[nb, p, mt, n] = input_blk[P*mt + p, 512*nb + n]
      out [NB, P, MT, 512] f32     : out[nb, p, mt, n] = result[P*mt + p, 512*nb + n]
    """
    KT = K // KP
    MT = m_shard // P
    NB = n_shard // NB_TILE
    assert NB == 4, "DMA ring layout below is written for NB == 4"

    nc = bacc.Bacc("TRN2", target_bir_lowering=False, debug=False)
    m1d = nc.dram_tensor("m1", [KT, P, MT, 2, P], F8, kind="ExternalInput")
    m2d = nc.dram_tensor("m2", [NB, P, KT, 2, NB_TILE], F8, kind="ExternalInput")
    inpd = nc.dram_tensor("inp", [NB, P, MT, NB_TILE], F8, kind="ExternalInput")
    outd = nc.dram_tensor("out", [NB, P, MT, NB_TILE], F32, kind="ExternalOutput")

    with tile.TileContext(nc) as tc:
        with (
            tc.tile_pool(name="m1", bufs=1) as m1_pool,
            tc.tile_pool(name="m2", bufs=4) as m2_pool,
            tc.tile_pool(name="inp", bufs=4) as inp_pool,
            tc.tile_pool(name="res", bufs=2) as res_pool,
            tc.tile_pool(name="psum", bufs=8, space="PSUM") as psum_pool,
        ):
            # PE warmup on a zeroed tile: keeps the HAM activity window busy
            # from ~7.4 us so the clock is ramping while the first DMAs land.
            warm_src = inp_pool.tile([P, P], F8, tag="warm", bufs=1)
            nc.gpsimd.memset(warm_src[:], 0.0)
            warm_ps = psum_pool.tile([P, NB_TILE], F32, tag="ps")
            for _ in range(N_WARMUP):
                nc.tensor.matmul(
                    warm_ps[:, :P], warm_src[:], warm_src[:], start=True, stop=True
                )

            # A DMA ring delivers strictly in queue order and the two rings
            # split bandwidth ~evenly with no QoS, so eager prefetch on one
            # ring starves urgent data on the other. Therefore: the Sync
            # ring alone (~360 GB/s, > the 222 GB/s the kt sweeps consume)
            # carries everything needed up to nb=2, in exact consumption
            # order; the Scalar ring is *gated* behind nb=0's first
            # eviction (its head instruction waits that add's semaphore),
            # then carries the late bulk (inp1..3, m2[3]) and the tail
            # store.
            m1s = m1_pool.tile([P, KT, MT, 2, P], F8, name="m1s", tag="m1", bufs=1)
            m2_tiles = [
                m2_pool.tile([P, KT, 2, NB_TILE], F8, tag="m2", name=f"m2s{nb}")
                for nb in range(NB)
            ]
            inps_all = [
                inp_pool.tile([P, MT, NB_TILE], F8, tag="inp", name=f"inps{nb}")
                for nb in range(NB)
            ]
            # kt=0 of m1 in two half-slices so the first matmuls gate on
            # 128 KiB; then strict need-order: sweep k consumes (m1[k], m2[k]).
            nc.sync.dma_start(m1s[:, 0, : MT // 2], m1d[0, :, : MT // 2])
            nc.sync.dma_start(m1s[:, 0, MT // 2 :], m1d[0, :, MT // 2 :])
            nc.sync.dma_start(m2_tiles[0][:, 0:1], m2d[0, :, 0:1])
            nc.sync.dma_start(m2_tiles[0][:, 1:2], m2d[0, :, 1:2])
            for kt in range(1, KT):
                nc.sync.dma_start(m1s[:, kt], m1d[kt])
                if kt % 2 == 0:
                    nc.sync.dma_start(
                        m2_tiles[0][:, kt : kt + 2], m2d[0, :, kt : kt + 2]
                    )
            nc.sync.dma_start(inps_all[0][:], inpd[0])
            for k0 in range(0, KT, 2):
                nc.sync.dma_start(m2_tiles[1][:, k0 : k0 + 2], m2d[1, :, k0 : k0 + 2])
            nc.sync.dma_start(m2_tiles[2][:], m2d[2])

            def evict(nb, mt, pss, inps, outs):
                """PSUM -> SBUF (+input) on DVE, then store."""
                nc.vector.tensor_add(outs[:, mt], pss[mt][:], inps[:, mt])
                eng = nc.scalar if (nb == 0 and mt == 0) else nc.sync
                eng.dma_start(outd[nb, :, mt], outs[:, mt])
                if nb == 0 and mt == 0:
                    # Now that the Scalar ring's head is gated (the store
                    # above waits on the add), queue the late bulk on it.
                    for nb2 in range(1, NB):
                        nc.scalar.dma_start(inps_all[nb2][:], inpd[nb2])
                    nc.scalar.dma_start(m2_tiles[3][:], m2d[3])

            # ---- nb = 0..NB-2: kt-outer / mt-inner -------------------------
            # One kt sweep = 8 MMs consuming one m1 kt-slice + one m2
            # kt-slice (384 KiB / 1.73 us = 222 GB/s), a pace the rings
            # sustain from cold; the stream starts as soon as (m1[kt=0],
            # m2[kt=0]) land. All 8 PSUM banks accumulate across the kt
            # loop; at an nb boundary the evictions chase the final kt
            # sweep m-tile by m-tile, so the next nb's matmuls never wait.
            for nb in range(NB - 1):
                m2s = m2_tiles[nb]
                pss = [psum_pool.tile([P, NB_TILE], F32, name=f"ps{nb}_{mt}", tag="ps")
                       for mt in range(MT)]
                outs = res_pool.tile([P, MT, NB_TILE], F32)
                for kt in range(KT):
                    for mt in range(MT):
                        nc.tensor.matmul(
                            pss[mt][:],
                            m1s[:, kt, mt],
                            m2s[:, kt],
                            start=(kt == 0),
                            stop=(kt == KT - 1),
                            perf_mode=mybir.MatmulPerfMode.DoubleRow,
                        )
                for mt in range(MT):
                    evict(nb, mt, pss, inps_all[nb], outs)

            # ---- nb = NB-1: mt-outer (m2 long prefetched) ------------------
            if True:
                nb = NB - 1
                m2s = m2_tiles[nb]
                inps = inps_all[nb]
                outs = res_pool.tile([P, MT, NB_TILE], F32)
                pss = [psum_pool.tile([P, NB_TILE], F32, name=f"ps{nb}_{mt}", tag="ps")
                       for mt in range(MT - 1)]
                last_mt = MT - 1
                for mt in range(MT - 1):
                    for kt in range(KT):
                        nc.tensor.matmul(
                            pss[mt][:],
                            m1s[:, kt, mt],
                            m2s[:, kt],
                            start=(kt == 0),
                            stop=(kt == KT - 1),
                            perf_mode=mybir.MatmulPerfMode.DoubleRow,
                        )
                    evict(nb, mt, pss, inps, outs)
                # Final m-tile: column-split across two PSUM banks. Half A
                # (cols 0..255) finishes 16 MMs early and its add+store
                # (Sync) overlap half B's matmuls; after the very last MM
                # only B's [128,256] add + one Scalar-ring store remain.
                H = NB_TILE // 2
                psA = psum_pool.tile([P, H], F32, name="psA", tag="ps")
                psB = psum_pool.tile([P, H], F32, name="psB", tag="ps")
                for ps, h in ((psA, 0), (psB, 1)):
                    cs = slice(h * H, (h + 1) * H)
                    for kt in range(KT):
                        nc.tensor.matmul(
                            ps[:],
                            m1s[:, kt, last_mt],
                            m2s[:, kt, :, cs],
                            start=(kt == 0),
                            stop=(kt == KT - 1),
                            perf_mode=mybir.MatmulPerfMode.DoubleRow,
                        )
                    nc.vector.tensor_add(
                        outs[:, last_mt, cs], ps[:], inps[:, last_mt, cs]
                    )
                    eng = nc.sync if h == 0 else nc.scalar
                    eng.dma_start(outd[nb, :, last_mt, cs], outs[:, last_mt, cs])
    nc.compile()
    return nc


def pack_m1_block(blk: np.ndarray) -> np.ndarray:
    """[m_shard, K] int -> [KT, P, MT, 2, P] fp8 (kt-major DoubleRow weights)."""
    m_shard, K = blk.shape
    # [mt, m, kt, i, p] from blk[P*mt + m, KP*kt + 128*i + p]
    r = blk.reshape(m_shard // P, P, K // KP, 2, P)
    return np.ascontiguousarray(r.transpose(2, 4, 0, 3, 1)).astype(np.float32).astype(
        ml_dtypes.float8_e4m3
    )


def pack_m2(mat2: np.ndarray) -> np.ndarray:
    """[K, N] int -> [N//512, P, KT, 2, 512] fp8 (DoubleRow moving layout)."""
    K, N = mat2.shape
    r = mat2.reshape(K // KP, 2, P, N // NB_TILE, NB_TILE)  # [kt, i, p, nb, n]
    return np.ascontiguousarray(r.transpose(3, 2, 0, 1, 4)).astype(np.float32).astype(
        ml_dtypes.float8_e4m3
    )


def pack_inp_block(blk: np.ndarray) -> np.ndarray:
    """[m_shard, n_shard] int -> [NB, P, MT, 512] fp8 (0..15 are exact)."""
    m_shard, n_shard = blk.shape
    r = blk.reshape(m_shard // P, P, n_shard // NB_TILE, NB_TILE)  # [mt, p, nb, n]
    return (
        np.ascontiguousarray(r.transpose(2, 1, 0, 3))
        .astype(np.float32)
        .astype(ml_dtypes.float8_e4m3)
    )


def unpack_out(packed: np.ndarray, m_shard: int, n_shard: int) -> np.ndarray:
    """[NB, P, MT, 512] f32 -> [m_shard, n_shard] f32."""
    return np.ascontiguousarray(packed.transpose(2, 1, 0, 3)).reshape(m_shard, n_shard)


def _prepare(input_tensor, mat1, mat2):
    input_tensor = np.asarray(input_tensor)
    mat1 = np.asarray(mat1)
    mat2 = np.asarray(mat2)
    M, K = mat1.shape
    N = mat2.shape[1]
    m_shard = M // A_SHARD
    n_shard = N // B_SHARD
    nb_per_core = n_shard // NB_TILE

    nc = build_program(m_shard, K, n_shard)

    m2p = pack_m2(mat2)  # [N//512, P, KT, 2, 512]; core takes its nb range
    in_maps = []
    for c in range(N_CORES):
        ra, cb = divmod(c, B_SHARD)
        rows = slice(ra * m_shard, (ra + 1) * m_shard)
        cols = slice(cb * n_shard, (cb + 1) * n_shard)
        nbs = slice(cb * nb_per_core, (cb + 1) * nb_per_core)
        in_maps.append(
            {
                "m1": pack_m1_block(mat1[rows]),
                "m2": m2p[nbs],
                "inp": pack_inp_block(input_tensor[rows, cols]),
            }
        )
    return nc, in_maps, (m_shard, n_shard)


def _gather(results, m_shard, n_shard):
    M = m_shard * A_SHARD
    N = n_shard * B_SHARD
    out = np.empty((M, N), dtype=np.float32)
    for c in range(N_CORES):
        ra, cb = divmod(c, B_SHARD)
        out[
            ra * m_shard : (ra + 1) * m_shard, cb * n_shard : (cb + 1) * n_shard
        ] = unpack_out(results[c]["out"], m_shard, n_shard)
    return out


def kernel(input_tensor, mat1, mat2):
    nc, in_maps, (m_shard, n_shard) = _prepare(input_tensor, mat1, mat2)
    res = run_bass_kernel_spmd(nc, in_maps, list(range(N_CORES))).results
    return _gather(res, m_shard, n_shard)


def kernel_traced(input_tensor, mat1, mat2, **kwargs):
    """Like kernel(), but also returns BassKernelResults (exec_time_ns etc.)."""
    nc, in_maps, (m_shard, n_shard) = _prepare(input_tensor, mat1, mat2)
    res = run_bass_kernel_spmd(
        nc, in_maps, list(range(N_CORES)), trace=True, **kwargs
    )
    return _gather(res.results, m_shard, n_shard), res
